# revision 24
# baseline (speedup 1.0000x reference)
"""Two-layer GCN (message passing) on 8 Trainium2 NeuronCores.

Strategy (v7 — gather-limited pipeline):
  - Uses linearity: A@(x@W1) = (A@x)@W1, so layer 1 gathers straight from
    the (bf16) input x table that every core already holds — no sharded
    GEMM, no first AllGather.  Likewise layer 2 aggregates relu(h1) rows
    and applies W2 after aggregation.
  - Dst nodes sharded across 8 cores (12500 each, 98 blocks of 128 slots,
    14 superblocks of 7 blocks).  Edges partitioned by dst owner; per core
    each (dst-block, src-group) pair gets a fixed 640-slot run.  Separate
    slot layouts per layer because the two gather tables index differently.
  - Weighted one-hot matrices are prebuilt on the host and streamed in by
    DMA (4.6 MB/superblock) — no DVE work, and no SBUF-port contention
    between the DVE and the Q7 descriptor generators (the gather
    bottleneck).
  - The h table for layer 2 is chunk-major ([chunk][core][rows]) so the
    AllGather runs as 6 chunk collectives {4,4,2,2,1,1} sbs that overlap
    layer-1 compute; gather windows (int16 limit) are independent 25088-row
    slices.  Layer 2's first two superblocks interleave their group-0..2
    gathers with the last chunk collective to hide its latency.
  - Per superblock: 4 dma_gather calls (queue 0-3) -> PE matmul
    scatter-add (msg^T @ onehot) into PSUM -> per-block GEMM with W1
    (+relu) or W2 -> DMA out.
"""

import os
import sys

import numpy as np

for _p in ("/opt/trn_rl_repo", "/root/.axon_site/_ro/trn_rl_repo"):
    if os.path.isdir(_p) and _p not in sys.path:
        sys.path.append(_p)

import ml_dtypes  # noqa: E402

import concourse.bacc as bacc  # noqa: E402
import concourse.mybir as mybir  # noqa: E402
from concourse import library_config, tile  # noqa: E402
from concourse.bass_utils import run_bass_kernel_spmd  # noqa: E402

BF16 = ml_dtypes.bfloat16

# ---- problem constants (nn_BaselineGCN: N=100000, E=1600000, 128->128->64) ----
N_NODES = 100000
N_EDGES = 1600000
F_IN = 128
F_HID = 128
F_OUT = 64

NCORES = 8
NPC = N_NODES // NCORES          # 12500 dst nodes per core
BLK = 128                        # nodes per dst block
NBLK = (NPC + BLK - 1) // BLK    # 98 blocks per core
SLOTPC = NBLK * BLK              # 12544 node slots per core (44 dummies)
NG = 4                           # src groups (int16 index limit)
G1 = N_NODES // NG               # 25000 rows per L1 gather group
CPB_G = 5                        # chunks per (block, group) run
RUNSLOTS = CPB_G * BLK           # 640 edge slots per run
KBLK = CPB_G * NG                # 20 chunks per block
SBB = 7                          # blocks per superblock
NSB = NBLK // SBB                # 14 superblocks
CH_SB = SBB * KBLK               # 140 chunks per superblock
SLOT_SB = CH_SB * 128            # 17920 edge slots per superblock
NCHUNK = NBLK * KBLK             # 1960 chunks per core per layer
NSLOT = NCHUNK * 128             # 250880 edge slots per core per layer
ROWS_SB = NCORES * SBB * BLK     # 7168 h2-table rows per superblock
H2ROWS = NSB * ROWS_SB           # 100352 rows in h2 table
G2 = H2ROWS // NG                # 25088 rows per L2 gather window
# AllGather chunks (in superblocks): table is chunk-major, rank-major inside
# each chunk so every chunk collective's output is contiguous.
CC_SBS = (4, 4, 2, 2, 1, 1)
CC_SB0 = (0, 4, 8, 10, 12, 13)
CC_BASE = tuple(NCORES * SBB * BLK * sb0 for sb0 in CC_SB0)
IDXCOLS = NSLOT // 16            # idx16 tensor free dim
IDXCOLS_SB = SLOT_SB // 16       # 1120 per superblock
IDXCOLS_G = RUNSLOTS * SBB // 16  # 280 idx cols per (superblock, group) call
NIDX_CALL = RUNSLOTS * SBB       # 4480 indices per gather call

_CACHE: dict = {}


def _wrap_idx16(v: np.ndarray) -> np.ndarray:
    """Pack indices for dma_gather: index i -> [i%16, i//16], replicated
    across the 8 groups of 16 partitions."""
    block = v.astype(np.int16).reshape(-1, 16).T  # [16, n/16]
    return np.tile(block, (8, 1))                 # [128, n/16]


def _chunk_major(a: np.ndarray, k: int) -> np.ndarray:
    return np.ascontiguousarray(
        a.reshape(NCHUNK, 128, k).transpose(1, 0, 2).reshape(
            128, NCHUNK * k)).astype(BF16)


def _layout(c: int, b: np.ndarray, g: np.ndarray, loc: np.ndarray,
            w: np.ndarray, idxval: np.ndarray):
    """Assign each edge a slot in the fixed (block, group)-run layout and
    build the packed idx16 / prebuilt one-hot tensors for one layer."""
    run = b * NG + g
    counts = np.bincount(run, minlength=NBLK * NG)
    if counts.max() > RUNSLOTS:
        raise RuntimeError(
            f"core {c}: run overflow {counts.max()} > {RUNSLOTS}; "
            f"increase CPB_G")

    order = np.argsort(run, kind="stable")
    run_s = run[order]
    start_of_run = np.searchsorted(run_s, np.arange(NBLK * NG))
    pos = np.arange(len(b)) - start_of_run[run_s]
    bs, gs = b[order], g[order]
    run_base = (bs // SBB) * SLOT_SB + gs * (SBB * RUNSLOTS) + (bs % SBB) * RUNSLOTS
    slot = run_base + pos

    idx = np.zeros(NSLOT, np.int64)
    idx[slot] = idxval[order]
    oh = np.zeros((NSLOT, 128), np.float32)
    oh[slot, loc[order]] = w[order]

    return _wrap_idx16(idx), _chunk_major(oh, 128)


def _prep_core(c: int, src: np.ndarray, dst: np.ndarray, ew: np.ndarray):
    """Per-core edge-slot layouts for both layers."""
    m = (dst // NPC) == c
    es = src[m].astype(np.int64)
    ed = (dst[m] - c * NPC).astype(np.int64)
    w = ew[m].astype(np.float32)

    b = ed // BLK
    loc = ed % BLK

    # layer 1: gather from x table [100000, 128]
    g1 = es // G1
    idx1, oh1 = _layout(c, b, g1, loc, w, es - g1 * G1)

    # layer 2: gather from chunk-major h table
    # [chunk][core][sbs-in-chunk * 896 rows]
    o = es // NPC
    l = es - o * NPC
    bsrc = l // BLK
    sb_s = bsrc // SBB
    ck = np.searchsorted(np.asarray(CC_SB0), sb_s, side="right") - 1
    row2 = (np.asarray(CC_BASE)[ck] + o * (np.asarray(CC_SBS)[ck] * SBB * BLK)
            + (sb_s - np.asarray(CC_SB0)[ck]) * (SBB * BLK)
            + (bsrc % SBB) * BLK + (l % BLK))
    g2 = row2 // G2
    idx2, oh2 = _layout(c, b, g2, loc, w, row2 - g2 * G2)

    return idx1, oh1, idx2, oh2


def _build_program():
    dbg_nsb = int(os.environ.get("KERNEL_DBG_NSB", str(NSB)))
    dbg_nogather = bool(int(os.environ.get("KERNEL_DBG_NOGATHER", "0")))
    dbg_nocoll = bool(int(os.environ.get("KERNEL_DBG_NOCOLL", "0")))
    scratch = int(os.environ.get("KERNEL_DMA_SCRATCH", "49152"))
    nc = bacc.Bacc("TRN2", target_bir_lowering=False, debug=False,
                   num_devices=NCORES, num_swdge_queues=4,
                   dynamic_dma_scratch_size=scratch)

    xtab_d = nc.dram_tensor("xtab", [N_NODES, F_IN], mybir.dt.bfloat16,
                            kind="ExternalInput")
    W1_d = nc.dram_tensor("W1b", [F_IN, F_HID], mybir.dt.bfloat16,
                          kind="ExternalInput")
    W2_d = nc.dram_tensor("W2b", [F_HID, F_OUT], mybir.dt.bfloat16,
                          kind="ExternalInput")
    idx1_d = nc.dram_tensor("idx1", [128, IDXCOLS], mybir.dt.int16,
                            kind="ExternalInput")
    idx2_d = nc.dram_tensor("idx2", [128, IDXCOLS], mybir.dt.int16,
                            kind="ExternalInput")
    oh1_d = nc.dram_tensor("oh1", [128, NCHUNK * 128], mybir.dt.bfloat16,
                           kind="ExternalInput")
    oh2_d = nc.dram_tensor("oh2", [128, NCHUNK * 128], mybir.dt.bfloat16,
                           kind="ExternalInput")
    out_d = nc.dram_tensor("out", [SLOTPC, F_OUT], mybir.dt.float32,
                           kind="ExternalOutput")

    with tile.TileContext(nc) as tc:
        nc.gpsimd.load_library(library_config.mlp)
        with (
            tc.tile_pool(name="dram", bufs=1, space="DRAM") as dram,
            tc.tile_pool(name="const", bufs=1) as constp,
            tc.tile_pool(name="idxp", bufs=2) as idxp,
            tc.tile_pool(name="msgp", bufs=8) as msgp,
            tc.tile_pool(name="ohp", bufs=2) as ohp,
            tc.tile_pool(name="smallp", bufs=4) as smallp,
            tc.tile_pool(name="psagg", bufs=2, space="PSUM") as psagg,
            tc.tile_pool(name="psgemm", bufs=2, space="PSUM") as psgemm,
        ):
            h2_loc = dram.tile([SLOTPC, F_HID], mybir.dt.bfloat16)
            # NOT addr_space="Shared": the sim enforces single-writer on
            # Shared DRAM, and we write h2_full with chunked collectives.
            h2_full = dram.tile([H2ROWS, F_HID], mybir.dt.bfloat16)

            w1_t = constp.tile([F_IN, F_HID], mybir.dt.bfloat16)
            nc.sync.dma_start(w1_t[:], W1_d[:])
            w2_t = constp.tile([F_HID, F_OUT], mybir.dt.bfloat16)
            nc.sync.dma_start(w2_t[:], W2_d[:])

            def trigger_cc(ck):
                """AllGather chunk ck (triggered well after the data is ready
                so the trigger never stalls the gpsimd queue). The chunk's
                output region is contiguous: [core][sbs-in-chunk rows]."""
                r0 = CC_SB0[ck] * SBB * BLK
                r1 = r0 + CC_SBS[ck] * SBB * BLK
                if dbg_nocoll:
                    nc.sync.dma_start(
                        h2_full[CC_BASE[ck]:CC_BASE[ck] + (r1 - r0), :],
                        h2_loc[r0:r1, :])
                    return
                nc.gpsimd.collective_compute(
                    "AllGather",
                    mybir.AluOpType.bypass,
                    ins=[h2_loc[r0:r1, :].opt()],
                    outs=[h2_full[CC_BASE[ck]:
                                  CC_BASE[ck] + NCORES * (r1 - r0), :].opt()],
                    replica_groups=[list(range(NCORES))],
                )

            for layer in (1, 2):
                idx_d = idx1_d if layer == 1 else idx2_d
                oh_d = oh1_d if layer == 1 else oh2_d
                gsz = G1 if layer == 1 else G2
                table = xtab_d if layer == 1 else h2_full

                def load_idx(sb):
                    idx_t = idxp.tile([128, IDXCOLS_SB], mybir.dt.int16)
                    nc.sync.dma_start(
                        idx_t[:],
                        idx_d[:, sb * IDXCOLS_SB:(sb + 1) * IDXCOLS_SB])
                    return idx_t

                def load_oh(sb):
                    oh_t = ohp.tile([128, CH_SB, 128], mybir.dt.bfloat16)
                    nc.sync.dma_start(
                        oh_t.rearrange("p c f -> p (c f)"),
                        oh_d[:, sb * SLOT_SB:(sb + 1) * SLOT_SB])
                    return oh_t

                def issue_gather(idx_t, g):
                    msg_g = msgp.tile([128, CH_SB // NG, 128],
                                      mybir.dt.bfloat16)
                    if not dbg_nogather:
                        nc.gpsimd.dma_gather(
                            msg_g[:],
                            table[g * gsz:(g + 1) * gsz, :],
                            idx_t[:, g * IDXCOLS_G:(g + 1) * IDXCOLS_G],
                            NIDX_CALL, NIDX_CALL, 128,
                            single_packet=False, queue_num=g,
                        )
                    else:
                        nc.vector.memset(msg_g[:, 0, :], 0.0)
                    return msg_g

                def process(sb, msgs, oh_t):
                    psA = psagg.tile([128, 512], mybir.dt.float32, tag="psA")
                    psB = psagg.tile([128, 512], mybir.dt.float32, tag="psB")

                    def agg_slice(bi):
                        pst = psA if bi < 4 else psB
                        j = bi if bi < 4 else bi - 4
                        return pst[:, j * 128:(j + 1) * 128]

                    # g-major (chunks in gather order, so matmuls of group g
                    # start as soon as gather g lands). PSUM has_written
                    # clear on start=True is bank-wide, so exactly one start
                    # per bank per superblock; per-element has_written then
                    # makes each block-slice's first write an overwrite.
                    for g in range(NG):
                        for bi in range(SBB):
                            for k in range(CPB_G):
                                ch = g * (CH_SB // NG) + bi * CPB_G + k
                                nc.tensor.matmul(
                                    agg_slice(bi),
                                    msgs[g][:, bi * CPB_G + k, :],
                                    oh_t[:, ch, :],
                                    start=(g == 0 and k == 0 and bi in (0, 4)),
                                    stop=(g == NG - 1 and k == CPB_G - 1
                                          and bi in (3, 6)),
                                    skip_group_check=True,
                                )

                    for bi in range(SBB):
                        b = sb * SBB + bi
                        # agg (PSUM, [128 f, 128 d]) -> bf16 SBUF
                        ag_t = smallp.tile([128, 128], mybir.dt.bfloat16,
                                           tag="aggc")
                        nc.scalar.activation(
                            ag_t[:], agg_slice(bi),
                            mybir.ActivationFunctionType.Copy)
                        if layer == 1:
                            # h1 = relu(agg^T @ W1) [128 d, 128 j]
                            hps = psgemm.tile([128, F_HID], mybir.dt.float32,
                                              tag="gemm")
                            nc.tensor.matmul(hps[:], ag_t[:], w1_t[:],
                                             start=True, stop=True)
                            h2b = smallp.tile([128, F_HID], mybir.dt.bfloat16,
                                              tag="h2b")
                            nc.scalar.activation(
                                h2b[:], hps[:],
                                mybir.ActivationFunctionType.Relu)
                            # write on scalar HWDGE: keeps the sync queue free
                            # for next-sb input prefetch (sync is in-order;
                            # output writes there would gate the next gathers)
                            nc.scalar.dma_start(
                                h2_loc[b * BLK:(b + 1) * BLK, :], h2b[:])
                        else:
                            # out = agg^T @ W2 [128 d, 64]
                            ops = psgemm.tile([128, F_OUT], mybir.dt.float32,
                                              tag="gemm")
                            nc.tensor.matmul(ops[:], ag_t[:], w2_t[:],
                                             start=True, stop=True)
                            ot = smallp.tile([128, F_OUT], mybir.dt.float32,
                                             tag="outb")
                            nc.scalar.activation(
                                ot[:], ops[:],
                                mybir.ActivationFunctionType.Copy)
                            nc.scalar.dma_start(
                                out_d[b * BLK:(b + 1) * BLK, :], ot[:])

                # cc triggers: chunk ck fires right after superblock
                # cc_plan[ck]'s gathers — by then its data has drained, so
                # the trigger doesn't stall the queue. Chunks 4 (sb12) and
                # 5 (sb13) fire after the loop / inside L2's prologue.
                cc_after = {6: 0, 10: 1, 12: 2, 13: 3} if dbg_nsb == NSB else {}

                start_sb = 0
                if layer == 2 and dbg_nsb == NSB:
                    # prologue: interleave sb0/sb1's window-0..2 gathers with
                    # the last chunk collective (sb13's rows) so its latency
                    # hides behind useful descriptor generation.
                    idx0 = load_idx(0)
                    oh0 = load_oh(0)
                    msgs0 = [issue_gather(idx0, g) for g in range(3)]
                    if not dbg_nocoll:
                        trigger_cc(5)
                    idx1t = load_idx(1)
                    oh1t = load_oh(1)
                    msgs1 = [issue_gather(idx1t, g) for g in range(3)]
                    msgs0.append(issue_gather(idx0, 3))
                    msgs1.append(issue_gather(idx1t, 3))
                    process(0, msgs0, oh0)
                    process(1, msgs1, oh1t)
                    start_sb = 2

                for sb in range(start_sb, dbg_nsb):
                    idx_t = load_idx(sb)
                    oh_t = load_oh(sb)
                    msgs = [issue_gather(idx_t, g) for g in range(NG)]
                    process(sb, msgs, oh_t)
                    if layer == 1 and sb in cc_after:
                        trigger_cc(cc_after[sb])

                if layer == 1:
                    if dbg_nsb == NSB:
                        trigger_cc(4)
                        if dbg_nocoll:
                            trigger_cc(5)
                    else:
                        nc.sync.dma_start(
                            h2_full[:dbg_nsb * SBB * BLK, :],
                            h2_loc[:dbg_nsb * SBB * BLK, :])

    nc.compile()
    return nc


def kernel(x, W1, W2, edge_weight, edge_index):
    x = np.asarray(x)
    W1 = np.asarray(W1)
    W2 = np.asarray(W2)
    ew = np.asarray(edge_weight)
    ei = np.asarray(edge_index)
    src, dst = ei[0].astype(np.int64), ei[1].astype(np.int64)

    if "nc" not in _CACHE:
        _CACHE["nc"] = _build_program()
    nc = _CACHE["nc"]

    xtab = np.ascontiguousarray(x).astype(BF16)
    w1b = W1.astype(BF16)
    w2b = W2.astype(BF16)

    in_maps = []
    for c in range(NCORES):
        idx1, oh1, idx2, oh2 = _prep_core(c, src, dst, ew)
        in_maps.append({
            "xtab": xtab,
            "W1b": w1b,
            "W2b": w2b,
            "idx1": idx1,
            "idx2": idx2,
            "oh1": oh1,
            "oh2": oh2,
        })

    trace = bool(int(os.environ.get("KERNEL_TRACE", "0")))
    res = run_bass_kernel_spmd(nc, in_maps, core_ids=list(range(NCORES)),
                               trace=trace)
    _CACHE["last_result"] = res

    out = np.empty((N_NODES, F_OUT), np.float32)
    for c in range(NCORES):
        out[c * NPC:(c + 1) * NPC] = res.results[c]["out"][:NPC]
    return out
